# revision 25
# baseline (speedup 1.0000x reference)
"""Multi-head attention (causal, interleaved RoPE) on 8 TRN2 NeuronCores.

Sharding: core c = (batch b = c//4, head-group g = c%4). Each core computes
4 heads of one batch fully on-device (QKV proj + RoPE + causal attention +
partial Wo projection); host sums the 4 row-parallel Wo partials per batch.

Single pipelined schedule: ScalarE exp (1 elem/cycle/lane, ~92us for the
causal softmax) is the pacer; attention S^T runs as two concurrent K=64
row-tiled matmuls (tile_position via base partitions 0/64), PV uses the
[V|1] ones-column trick for softmax sums, and all matmul operands are bf16
(FWL stays enabled, no fp32-HIGH passes). proj/wo matmuls are drip-fed as
"filler" units into the per-iteration PE bubbles left by the exp latency.
PSUM budget: sps 2x2 banks + pv 2 banks + shared aux ring 2 banks = 8.
"""
import ml_dtypes
import numpy as np

import concourse.bass as bass
import concourse.mybir as mybir
import concourse.tile as tile
from concourse import bacc
from concourse.bass_utils import run_bass_kernel_spmd

f32 = mybir.dt.float32
bf16 = mybir.dt.bfloat16
AF = mybir.ActivationFunctionType

T, D = 2048, 1024
G = 4            # heads per core
NTS = 4          # t-slices of 512
TS = T // NTS    # 512
DCH = D // 128   # 8 contraction chunks
ROPE_BASE = 10000.0

_CACHE = {}


def _build():
    nc = bacc.Bacc(None, target_bir_lowering=False)
    xt = nc.dram_tensor("xt", [D, T], bf16, kind="ExternalInput")
    wqt = nc.dram_tensor("wqt", [D, 256], bf16, kind="ExternalInput")
    wkt = nc.dram_tensor("wkt", [D, 256], bf16, kind="ExternalInput")
    wvt = nc.dram_tensor("wvt", [D, 256], bf16, kind="ExternalInput")
    wot = nc.dram_tensor("wot", [256, D], bf16, kind="ExternalInput")
    cosp = nc.dram_tensor("cosp", [128, T], f32, kind="ExternalInput")
    sinp = nc.dram_tensor("sinp", [128, T], f32, kind="ExternalInput")
    triu = nc.dram_tensor("triu", [128, 128], bf16, kind="ExternalInput")
    outp = nc.dram_tensor("outp", [T, D], bf16, kind="ExternalOutput")

    xt_r = xt.rearrange("(dc p) t -> p dc t", p=128)
    wqt_r = wqt.rearrange("(dc p) j -> p dc j", p=128)
    wkt_r = wkt.rearrange("(dc p) j -> p dc j", p=128)
    wvt_r = wvt.rearrange("(dc p) j -> p dc j", p=128)
    wot_r = wot.rearrange("(c p) m -> p c m", p=128)
    outp_r = outp.rearrange("(tt p) m -> p tt m", p=128)

    with tile.TileContext(nc) as tc:
        with (
            tc.tile_pool(name="const", bufs=1) as const,
            tc.tile_pool(name="xtp", bufs=2) as xtp,
            tc.tile_pool(name="ut", bufs=2) as ut,
            tc.tile_pool(name="expp", bufs=4) as expp,
            tc.tile_pool(name="nrm", bufs=2) as nrm,
            tc.tile_pool(name="osb", bufs=4) as osb,
            tc.tile_pool(name="sps", bufs=2, space="PSUM") as sps,
            tc.tile_pool(name="pvp", bufs=1, space="PSUM") as pvp,
            tc.tile_pool(name="aux", bufs=2, space="PSUM") as aux,
        ):
            wq_sb = const.tile([128, DCH, 256], bf16)
            wk_sb = const.tile([128, DCH, 256], bf16)
            wv_sb = const.tile([128, DCH, 256], bf16)
            wo_sb = const.tile([128, 2, D], bf16)
            triu_sb = const.tile([128, 128], bf16)
            cos_sb = const.tile([128, T], f32)
            sin_sb = const.tile([128, T], f32)

            # per-slice persistent tensors (slice-tagged for dependency tracking)
            qTs = [const.tile([128, 2, TS], bf16, name=f"qT{i}", tag=f"qT{i}") for i in range(NTS)]
            kzs = [const.tile([128, 2, TS], bf16, name=f"kz{i}", tag=f"kz{i}") for i in range(NTS)]
            vss = [const.tile([128, 4, G, 65], bf16, name=f"v{i}", tag=f"v{i}") for i in range(NTS)]
            oTs = [const.tile([128, 2, TS], bf16, name=f"oT{i}", tag=f"oT{i}") for i in range(NTS)]
            for i in range(NTS):
                nc.vector.memset(vss[i][:, :, :, 64:65], 1.0)

            xt_sb = {}

            def load_xt(tsi):
                xts = xtp.tile([128, DCH, TS], bf16, tag="xts", name="xts")
                nc.sync.dma_start(xts[:], xt_r[:, :, tsi * TS:(tsi + 1) * TS])
                xt_sb[tsi] = xts

            def load_cs(tsi):
                sl = slice(tsi * TS, (tsi + 1) * TS)
                nc.sync.dma_start(cos_sb[:, sl], cosp[:, sl])
                nc.sync.dma_start(sin_sb[:, sl], sinp[:, sl])

            # DMA order = consumption order; first chunks split small so the
            # first matmul group's deps land in ~1us
            nc.sync.dma_start(wq_sb[:, 0:2, :], wqt_r[:, 0:2, :])
            xts0 = xtp.tile([128, DCH, TS], bf16, tag="xts", name="xts")
            nc.sync.dma_start(xts0[:, 0:2, :], xt_r[:, 0:2, 0:TS])
            nc.sync.dma_start(wq_sb[:, 2:, :], wqt_r[:, 2:, :])
            nc.sync.dma_start(xts0[:, 2:4, :], xt_r[:, 2:4, 0:TS])
            nc.sync.dma_start(cos_sb[:, 0:TS], cosp[:, 0:TS])
            nc.sync.dma_start(wk_sb[:, 0:4, :], wkt_r[:, 0:4, :])
            nc.sync.dma_start(xts0[:, 4:, :], xt_r[:, 4:, 0:TS])
            xt_sb[0] = xts0
            nc.sync.dma_start(sin_sb[:, 0:TS], sinp[:, 0:TS])
            nc.sync.dma_start(wk_sb[:, 4:, :], wkt_r[:, 4:, :])
            nc.sync.dma_start(wv_sb[:], wvt_r)
            load_xt(1)
            load_cs(1)
            nc.sync.dma_start(triu_sb[:], triu[:])
            load_cs(2)
            load_cs(3)

            # preload the exp activation table before the attn phase needs it
            scr = nrm.tile([1, 8], f32, tag="scr", name="scr")
            nc.scalar.activation(scr[:], cos_sb[0:1, 0:8], AF.Exp)

            # ---------------- filler machinery ----------------
            fillers = []

            def pump(n):
                for _ in range(n):
                    if not fillers:
                        return
                    fillers.pop(0)()

            def proj_qk_units(tsi, w_sb, is_q):
                """QKV projection for one of Wq/Wk: 2 t-chunks x 2 roles x 8
                contraction MMs (N=256) into a 1-bank aux PSUM tile, RoPE'd on
                DVE, relaid to qT/kz via partition-shuffling DMAs."""
                units = []
                xtd = xt_sb[tsi]
                dst = qTs[tsi] if is_q else kzs[tsi]
                state = {}

                def unit(c, r, du):
                    if r == 0 and du == 0:
                        state["ps"] = aux.tile([128, 2, 256], f32, tag="aux", name="ps")
                        if c == 0:
                            state["rt"] = ut.tile([128, 2, TS], bf16, tag="rt", name="rt")
                    ps = state["ps"]
                    for d in (2 * du, 2 * du + 1):
                        nc.tensor.matmul(
                            ps[:, r, :],
                            w_sb[:, d, r * 128:(r + 1) * 128],
                            xtd[:, d, c * 256:(c + 1) * 256],
                            start=(d == 0),
                            stop=(d == DCH - 1),
                        )
                    if r == 1 and du == 3:
                        csl = slice(tsi * TS + c * 256, tsi * TS + (c + 1) * 256)
                        rsl = slice(c * 256, (c + 1) * 256)
                        rt = state["rt"]
                        uc = ut.tile([128, 2, 256], bf16, tag="uc", name="uc")
                        us = ut.tile([128, 2, 256], bf16, tag="us", name="us")
                        nc.vector.tensor_mul(
                            uc[:], ps[:], cos_sb[:, None, csl].to_broadcast((128, 2, 256))
                        )
                        nc.vector.tensor_mul(
                            us[:], ps[:, ::-1, :], sin_sb[:, None, csl].to_broadcast((128, 2, 256))
                        )
                        nc.vector.tensor_sub(rt[:, 0, rsl], uc[:, 0, :], us[:, 0, :])
                        nc.vector.tensor_add(rt[:, 1, rsl], uc[:, 1, :], us[:, 1, :])
                        if c == 1:
                            # partition relayout: row h*32+f, role r ->
                            # row (h%2)*64 + r*32 + f, chunk h//2
                            for h in range(G):
                                hp, hh = h // 2, h % 2
                                for rr in (0, 1):
                                    nc.sync.dma_start(
                                        dst[hh * 64 + rr * 32:hh * 64 + (rr + 1) * 32, hp, :],
                                        rt[h * 32:(h + 1) * 32, rr, :],
                                    )

                for c in range(2):
                    for r in range(2):
                        for du in range(4):
                            units.append(lambda c=c, r=r, du=du: unit(c, r, du))
                return units

            def proj_v_units(tsi):
                units = []
                xtd = xt_sb[tsi]
                state = {}

                def unit(st, du):
                    if du == 0:
                        state["psv"] = aux.tile([128, 256], f32, tag="aux", name="psv")
                    psv = state["psv"]
                    for d in (2 * du, 2 * du + 1):
                        nc.tensor.matmul(
                            psv[:],
                            xtd[:, d, st * 128:(st + 1) * 128],
                            wv_sb[:, d, :],
                            start=(d == 0),
                            stop=(d == DCH - 1),
                        )
                    if du == 3:
                        nc.vector.tensor_copy(
                            vss[tsi][:, st, :, 0:64],
                            psv[:].rearrange("p (g dh) -> p g dh", g=G),
                        )

                for st in range(4):
                    for du in range(4):
                        units.append(lambda st=st, du=du: unit(st, du))
                return units

            def wo_units(qt, on_scalar=False):
                units = []

                def unit(t4, mh):
                    po = aux.tile([128, TS], f32, tag="aux", name="po")
                    for hc in (0, 1):
                        nc.tensor.matmul(
                            po[:],
                            oTs[qt][:, hc, t4 * 128:(t4 + 1) * 128],
                            wo_sb[:, hc, mh * TS:(mh + 1) * TS],
                            start=(hc == 0),
                            stop=(hc == 1),
                        )
                    ob = osb.tile([128, TS], bf16, tag="ob", name="ob")
                    if on_scalar and (t4 + mh) % 2 == 0:
                        nc.scalar.copy(ob[:], po[:])
                    else:
                        nc.vector.tensor_copy(ob[:], po[:])
                    nc.sync.dma_start(outp_r[:, qt * 4 + t4, mh * TS:(mh + 1) * TS], ob[:])

                for t4 in range(4):
                    for mh in (0, 1):
                        units.append(lambda t4=t4, mh=mh: unit(t4, mh))
                return units

            # ---------------- attention ----------------
            def attn(qt, pump_n):
                K = 4 * qt + 4
                for pair in (0, 1):
                    pv = [
                        pvp.tile([65, TS], f32, tag=f"pv{hh}", name=f"pv{hh}")
                        for hh in (0, 1)
                    ]
                    exs = {}
                    offs = {}

                    def S(ko):
                        off = max(0, ko - 4 * qt) * 128
                        offs[ko] = off
                        tko, kin = divmod(ko, 4)
                        ps_s = sps.tile([128, 2, TS], f32, tag="s", name="ps_s")
                        for hh in (0, 1):
                            # K=64 row-tiled pair: tile_position (hh*64, 0)
                            nc.tensor.matmul(
                                ps_s[:, hh, off:],
                                kzs[tko][hh * 64:(hh + 1) * 64, pair, kin * 128:(kin + 1) * 128],
                                qTs[qt][hh * 64:(hh + 1) * 64, pair, off:],
                                start=True,
                                stop=True,
                            )
                        ex = expp.tile([128, 2, TS], bf16, tag="ex", name="ex")
                        nc.scalar.activation(
                            ex[:, :, off:], ps_s[:, :, off:], AF.Exp, scale=0.125
                        )
                        if ko >= 4 * qt:
                            nc.vector.tensor_mul(
                                ex[:, :, off:off + 128],
                                ex[:, :, off:off + 128],
                                triu_sb[:, None, :].to_broadcast((128, 2, 128)),
                            )
                        exs[ko] = ex

                    def PV(ko):
                        off = offs[ko]
                        tko, kin = divmod(ko, 4)
                        for hh in (0, 1):
                            nc.tensor.matmul(
                                pv[hh][:, off:],
                                vss[tko][:, kin, 2 * pair + hh, :],
                                exs[ko][:, hh, off:],
                                start=(ko == 0),
                                stop=(ko == K - 1),
                            )
                        del exs[ko]

                    for ko in range(K):
                        S(ko)
                        if ko > 0:
                            PV(ko - 1)
                        pump(pump_n)
                    PV(K - 1)

                    for hh in (0, 1):
                        s0 = nrm.tile([1, TS], f32, tag="s0", name="s0")
                        nc.vector.tensor_copy(s0[:], pv[hh][64:65, :])
                        rc = nrm.tile([1, TS], f32, tag="rc", name="rc")
                        nc.vector.reciprocal_approx_fast(out=rc[:], in_=s0[:])
                        rb = nrm.tile([64, TS], f32, tag="rb", name="rb")
                        nc.gpsimd.partition_broadcast(rb[:], rc[:])
                        nc.vector.tensor_mul(
                            oTs[qt][hh * 64:(hh + 1) * 64, pair, :], pv[hh][0:64, :], rb[:]
                        )

            # ---------------- schedule ----------------
            for u in proj_qk_units(0, wq_sb, True):
                u()
            for u in proj_qk_units(0, wk_sb, False):
                u()
            for u in proj_v_units(0):
                u()

            fillers += proj_qk_units(1, wq_sb, True)
            fillers += proj_qk_units(1, wk_sb, False)
            fillers += proj_v_units(1)
            attn(0, 6)
            while fillers:          # drain proj(1) before attn(1) needs it
                fillers.pop(0)()
            load_xt(2)
            nc.sync.dma_start(wo_sb[:], wot_r)
            fillers += proj_qk_units(2, wq_sb, True)
            fillers += proj_qk_units(2, wk_sb, False)
            fillers += proj_v_units(2)
            attn(1, 3)
            load_xt(3)
            fillers += wo_units(0)
            fillers += proj_qk_units(3, wq_sb, True)
            fillers += proj_qk_units(3, wk_sb, False)
            attn(2, 2)
            fillers += proj_v_units(3)
            fillers += wo_units(1)
            fillers += wo_units(2)
            attn(3, 2)
            while fillers:
                fillers.pop(0)()
            # wo(3) tail: 2-bank sps tiles (free after the last exp), 4 MMs +
            # one wide copy (alternating scalar/DVE) + one DMA per t-chunk
            for t4 in range(4):
                po2 = sps.tile([128, 2, TS], f32, tag="s", name="po2")
                for mh in (0, 1):
                    for hc in (0, 1):
                        nc.tensor.matmul(
                            po2[:, mh, :],
                            oTs[3][:, hc, t4 * 128:(t4 + 1) * 128],
                            wo_sb[:, hc, mh * TS:(mh + 1) * TS],
                            start=(hc == 0),
                            stop=(hc == 1),
                        )
                ob2 = osb.tile([128, 2, TS], bf16, tag="ob2", name="ob2")
                if t4 % 2 == 0:
                    nc.scalar.copy(ob2[:], po2[:])
                else:
                    nc.vector.tensor_copy(ob2[:], po2[:])
                nc.sync.dma_start(
                    outp_r[:, 12 + t4, :], ob2[:].rearrange("p a b -> p (a b)")
                )
    nc.compile()
    return nc


def _get_nc():
    if "nc" not in _CACHE:
        _CACHE["nc"] = _build()
    return _CACHE["nc"]


def _host_inputs(x, Wq, Wk, Wv, Wo):
    """Build per-core input dicts (host-side sharding / layout prep)."""
    jj = np.arange(256)
    role = jj // 128
    h = (jj % 128) // 32
    f = jj % 32
    inv_freq = 1.0 / (ROPE_BASE ** (np.arange(0, 64, 2, dtype=np.float64) / 64.0))
    t = np.arange(T, dtype=np.float64)
    ang = t[None, :] * inv_freq[np.arange(128) % 32][:, None]   # [128, T]
    cosp = np.cos(ang).astype(np.float32)
    sinp = np.sin(ang).astype(np.float32)
    triu = (np.arange(128)[None, :] >= np.arange(128)[:, None]).astype(ml_dtypes.bfloat16)

    in_maps = []
    for core in range(8):
        b, g = divmod(core, 4)
        jsel = (g * 4 + h) * 64 + 2 * f + role
        in_maps.append({
            "xt": np.ascontiguousarray(x[b].T).astype(ml_dtypes.bfloat16),
            "wqt": np.ascontiguousarray(Wq[jsel, :].T).astype(ml_dtypes.bfloat16),
            "wkt": np.ascontiguousarray(Wk[jsel, :].T).astype(ml_dtypes.bfloat16),
            "wvt": np.ascontiguousarray(Wv[g * 256:(g + 1) * 256, :].T).astype(ml_dtypes.bfloat16),
            "wot": np.ascontiguousarray(Wo[:, g * 256:(g + 1) * 256].T).astype(ml_dtypes.bfloat16),
            "cosp": cosp,
            "sinp": sinp,
            "triu": triu,
        })
    return in_maps


def run(x, Wq, Wk, Wv, Wo, trace=False):
    nc = _get_nc()
    in_maps = _host_inputs(x, Wq, Wk, Wv, Wo)
    res = run_bass_kernel_spmd(nc, in_maps, core_ids=list(range(8)), trace=trace)
    out = np.zeros((2, T, D), dtype=np.float64)
    for core in range(8):
        out[core // 4] += res.results[core]["outp"].astype(np.float64)
    return out.astype(np.float32), res


def kernel(x=None, mask=None, Wq=None, Wk=None, Wv=None, Wo=None, **_ignored):
    x = np.asarray(x, dtype=np.float32)
    Wq = np.asarray(Wq, dtype=np.float32)
    Wk = np.asarray(Wk, dtype=np.float32)
    Wv = np.asarray(Wv, dtype=np.float32)
    Wo = np.asarray(Wo, dtype=np.float32)
    out, _ = run(x, Wq, Wk, Wv, Wo, trace=False)
    return out
